# revision 1
# baseline (speedup 1.0000x reference)
"""Trainium2 Bass kernel for a 2-layer GATv2 + GraphNorm node classifier.

Strategy (8 NeuronCores, SPMD single NEFF):
  - Nodes are sharded contiguously: core k owns nodes [k*6250, (k+1)*6250).
  - Host (index-only preprocessing): add self loops, route each edge to the
    owner of its dst, sort by dst, group into 128-node blocks, pad each block's
    edge list to a whole number of 128-edge tiles (schedule shared by all
    cores so one program serves all), remap src to a padded table index,
    pre-transpose x.
  - Device per core: build the full xl=x@Wl+bl gather table (replicated),
    local xr blocks; per 128-edge tile: one-hot(dst) matrix via is_equal,
    TensorE matmuls for xr broadcast + attention-dot + softmax-weighted
    scatter-add accumulated in PSUM per 128-node block.  GraphNorm is folded
    into the next layer's weights (stats via matmul + AllReduce[64,2]);
    layer-2 gather table built after AllGather of h1 (transposed layout).
  - Softmax max-subtraction is skipped (|e| bounded ~<10, exp is safe in f32).
"""

import numpy as np

import concourse.bacc as bacc
import concourse.bass as bass
import concourse.mybir as mybir
import concourse.tile as tile
from concourse.masks import make_identity

F32 = mybir.dt.float32
I32 = mybir.dt.int32
AF = mybir.ActivationFunctionType
OP = mybir.AluOpType

P = 128


class Cfg:
    def __init__(self, n_nodes, n_cores=8):
        assert n_nodes % n_cores == 0
        self.N = n_nodes
        self.NC = n_cores
        self.NPC = n_nodes // n_cores          # real nodes per core
        self.BLOCKS = (self.NPC + P - 1) // P  # 128-node blocks per core
        self.NPADC = self.BLOCKS * P           # padded nodes per core
        self.NPAD_ALL = self.NC * self.NPADC   # padded table rows
        self.DIN = 128
        self.HC = 128                          # H*C
        self.C = 64
        self.NCLS = 4
        self.EPS = 1e-5


def _preprocess(cfg, x, edge_index):
    """Host-side index preprocessing + input staging. Returns (T_list, in_maps_extra)."""
    N, NC, NPC, BLOCKS, NPADC = cfg.N, cfg.NC, cfg.NPC, cfg.BLOCKS, cfg.NPADC
    E = edge_index.shape[1]
    src = np.concatenate([edge_index[0].astype(np.int64), np.arange(N, dtype=np.int64)])
    dst = np.concatenate([edge_index[1].astype(np.int64), np.arange(N, dtype=np.int64)])

    core = dst // NPC
    dloc = dst - core * NPC
    blk = dloc // P
    dstl = dloc - blk * P                      # within-block dst index [0,128)
    gb = core * BLOCKS + blk                   # global (core, block) id

    # per-(core,block) counts -> shared tile schedule
    cnt = np.bincount(gb, minlength=NC * BLOCKS).reshape(NC, BLOCKS)
    T_list = np.maximum(1, (cnt.max(axis=0) + P - 1) // P).astype(np.int64)  # [BLOCKS]
    T_total = int(T_list.sum())
    offs = np.concatenate([[0], np.cumsum(T_list)])  # tile offset per block

    srcr = (src // NPC) * NPADC + (src % NPC)  # remapped src (padded table row)

    esrcT = np.zeros((NC, P, T_total), dtype=np.int32)
    edstlT = np.full((NC, P, T_total), -1.0, dtype=np.float32)

    order = np.lexsort((dstl, gb))  # sort edges by (core, block) then dstl (any in-block order ok)
    gb_s, dstl_s, srcr_s = gb[order], dstl[order], srcr[order]
    # slot position of each edge within its (core, block) group
    pos_in_group = np.arange(len(gb_s)) - np.searchsorted(gb_s, gb_s, side="left")
    core_s = gb_s // BLOCKS
    blk_s = gb_s % BLOCKS
    slot = offs[blk_s] * P + pos_in_group      # flat slot inside this core's edge array
    tile_i = slot // P
    part_i = slot % P
    esrcT[core_s, part_i, tile_i] = srcr_s
    edstlT[core_s, part_i, tile_i] = dstl_s.astype(np.float32)

    # transposed, padded x
    xT = np.zeros((cfg.DIN, cfg.NPAD_ALL), dtype=np.float32)
    xsrc = np.ascontiguousarray(x.T)  # [DIN, N]
    for k in range(NC):
        xT[:, k * NPADC : k * NPADC + NPC] = xsrc[:, k * NPC : (k + 1) * NPC]

    per_core = []
    for k in range(NC):
        per_core.append({
            "xT": np.ascontiguousarray(xT),
            "xTl": np.ascontiguousarray(xT[:, k * NPADC : (k + 1) * NPADC]),
            "esrcT": np.ascontiguousarray(esrcT[k]),
            "edstlT": np.ascontiguousarray(edstlT[k]),
        })
    return [int(t) for t in T_list], per_core


def _build(cfg, T_list):
    """Build + compile the SPMD program. Returns nc."""
    NC, BLOCKS, NPADC, NPAD_ALL = cfg.NC, cfg.BLOCKS, cfg.NPADC, cfg.NPAD_ALL
    NPC, HC, C, NCLS = cfg.NPC, cfg.HC, cfg.C, cfg.NCLS
    T_total = sum(T_list)
    NT = NC * BLOCKS
    rg = [list(range(NC))]
    LAST = NPC - (BLOCKS - 1) * P  # real rows in last block

    nc = bacc.Bacc("TRN2", target_bir_lowering=False, debug=False,
                   enable_asserts=False, num_devices=NC)

    # ---------------- IO ----------------
    xT_d = nc.dram_tensor("xT", [128, NPAD_ALL], F32, kind="ExternalInput")
    xTl_d = nc.dram_tensor("xTl", [128, NPADC], F32, kind="ExternalInput")
    esrcT_d = nc.dram_tensor("esrcT", [P, T_total], I32, kind="ExternalInput")
    edstlT_d = nc.dram_tensor("edstlT", [P, T_total], F32, kind="ExternalInput")
    w = {}
    for li, din in ((1, 128), (2, 64)):
        w[f"Wl{li}"] = nc.dram_tensor(f"Wl{li}", [din, HC], F32, kind="ExternalInput")
        w[f"Wr{li}"] = nc.dram_tensor(f"Wr{li}", [din, HC], F32, kind="ExternalInput")
        w[f"bl{li}"] = nc.dram_tensor(f"bl{li}", [HC], F32, kind="ExternalInput")
        w[f"br{li}"] = nc.dram_tensor(f"br{li}", [HC], F32, kind="ExternalInput")
        w[f"att{li}"] = nc.dram_tensor(f"att{li}", [2, C], F32, kind="ExternalInput")
        w[f"bias{li}"] = nc.dram_tensor(f"bias{li}", [C], F32, kind="ExternalInput")
        w[f"gng{li}"] = nc.dram_tensor(f"gng{li}", [C], F32, kind="ExternalInput")
        w[f"gnb{li}"] = nc.dram_tensor(f"gnb{li}", [C], F32, kind="ExternalInput")
        w[f"gna{li}"] = nc.dram_tensor(f"gna{li}", [C], F32, kind="ExternalInput")
    W1_d = nc.dram_tensor("W1", [C, NCLS], F32, kind="ExternalInput")
    b1_d = nc.dram_tensor("b1", [NCLS], F32, kind="ExternalInput")
    out_d = nc.dram_tensor("out", [NPC, NCLS], F32, kind="ExternalOutput")
    import os as _os
    DBG = bool(int(_os.environ.get("GAT_DEBUG", "0")))
    if DBG:
        dbg_xl1 = nc.dram_tensor("dbg_xl1", [256, HC], F32, kind="ExternalOutput")
        dbg_h1T = nc.dram_tensor("dbg_h1T", [C, NPADC], F32, kind="ExternalOutput")
        dbg_st = nc.dram_tensor("dbg_st", [C, 2], F32, kind="ExternalOutput")
        dbg_xr1 = nc.dram_tensor("dbg_xr1", [P, HC], F32, kind="ExternalOutput")
        dbg_AB = nc.dram_tensor("dbg_AB", [C, 2], F32, kind="ExternalOutput")
        dbg_ag = nc.dram_tensor("dbg_ag", [C * NC, 128], F32, kind="ExternalOutput")
        dbg_xl2 = nc.dram_tensor("dbg_xl2", [256, HC], F32, kind="ExternalOutput")
        dbg_xr2 = nc.dram_tensor("dbg_xr2", [P, HC], F32, kind="ExternalOutput")
        dbg_h2T = nc.dram_tensor("dbg_h2T", [C, NPADC], F32, kind="ExternalOutput")

    # ---------------- internal DRAM ----------------
    xl1_t = nc.dram_tensor("xl1_t", [NPAD_ALL, HC], F32, kind="Internal")
    xl2_t = nc.dram_tensor("xl2_t", [NPAD_ALL, HC], F32, kind="Internal")
    h1T_dr = nc.dram_tensor("h1T_dr", [C, NPADC], F32, kind="Internal")
    h1T_ag = nc.dram_tensor("h1T_ag", [C * NC, NPADC], F32, kind="Internal", addr_space="Shared")
    st_l = [nc.dram_tensor(f"st{li}_l", [C, 2], F32, kind="Internal") for li in (1, 2)]
    st_g = [nc.dram_tensor(f"st{li}_g", [C, 2], F32, kind="Internal", addr_space="Shared") for li in (1, 2)]

    with tile.TileContext(nc) as tc:
        import contextlib
        ctx = contextlib.ExitStack()
        with ctx:
            con = ctx.enter_context(tc.tile_pool(name="con", bufs=1))
            res = ctx.enter_context(tc.tile_pool(name="res", bufs=1))
            sb = ctx.enter_context(tc.tile_pool(name="sb", bufs=4))
            sgath = ctx.enter_context(tc.tile_pool(name="sgath", bufs=6))
            sidx = ctx.enter_context(tc.tile_pool(name="sidx", bufs=2))
            ps_t = ctx.enter_context(tc.tile_pool(name="ps_t", bufs=1, space="PSUM"))
            ps_b = ctx.enter_context(tc.tile_pool(name="ps_b", bufs=2, space="PSUM"))
            ps_e = ctx.enter_context(tc.tile_pool(name="ps_e", bufs=1, space="PSUM"))
            ps_pet = ctx.enter_context(tc.tile_pool(name="ps_pet", bufs=1, space="PSUM"))
            ps_acc = ctx.enter_context(tc.tile_pool(name="ps_acc", bufs=2, space="PSUM"))
            ps_st = ctx.enter_context(tc.tile_pool(name="ps_st", bufs=1, space="PSUM"))

            # ---------------- constants ----------------
            ident = con.tile([P, P], F32)
            make_identity(nc, ident[:])
            iota_i = con.tile([P, P], I32)
            nc.gpsimd.iota(iota_i[:], pattern=[[1, P]], base=0, channel_multiplier=0)
            iota_f = con.tile([P, P], F32)
            nc.vector.tensor_copy(iota_f[:], iota_i[:])
            ones_col = con.tile([P, 1], F32)
            nc.vector.memset(ones_col[:], 1.0)
            ones_row = con.tile([1, P], F32)
            nc.vector.memset(ones_row[:], 1.0)
            # mask column: 1.0 for real rows of the last node block, 0 for pad
            mask_col = con.tile([P, 1], F32)
            nc.vector.memset(mask_col[:], 1.0)
            if LAST < P:
                nc.gpsimd.affine_select(
                    out=mask_col[:], in_=mask_col[:], compare_op=OP.is_ge,
                    fill=0.0, base=LAST - 1, channel_multiplier=-1, pattern=[[0, 1]])

            def load_row(d, n):  # [n] dram -> [1, n] sbuf
                t = con.tile([1, n], F32, tag=f"row_{d.name}")
                nc.sync.dma_start(out=t[:], in_=d[None, :])
                return t

            def load_col(d, n):  # [n] dram -> [n, 1] sbuf
                t = con.tile([n, 1], F32, tag=f"col_{d.name}")
                nc.sync.dma_start(out=t[:], in_=d[:, None])
                return t

            def replicate_row(row_t, n, tag):  # [1,n] -> [P,n]
                pr = ps_b.tile([P, n], F32, space="PSUM", tag="ps_mm")
                nc.tensor.matmul(pr[:], lhsT=ones_row[:], rhs=row_t[:], start=True, stop=True)
                t = con.tile([P, n], F32, tag=tag)
                nc.scalar.copy(t[:], pr[:])
                return t

            Wl1_sb = con.tile([128, HC], F32)
            nc.sync.dma_start(out=Wl1_sb[:], in_=w["Wl1"][:])
            Wr1_sb = con.tile([128, HC], F32)
            nc.sync.dma_start(out=Wr1_sb[:], in_=w["Wr1"][:])
            Wl2_sb = con.tile([C, HC], F32)
            nc.sync.dma_start(out=Wl2_sb[:], in_=w["Wl2"][:])
            Wr2_sb = con.tile([C, HC], F32)
            nc.sync.dma_start(out=Wr2_sb[:], in_=w["Wr2"][:])
            W1_sb = con.tile([C, NCLS], F32)
            nc.sync.dma_start(out=W1_sb[:], in_=W1_d[:])
            b1_row = load_row(b1_d, NCLS)

            bl1_rep = replicate_row(load_row(w["bl1"], HC), HC, "bl1_rep")
            br1_rep = replicate_row(load_row(w["br1"], HC), HC, "br1_rep")
            bias_rep = [replicate_row(load_row(w[f"bias{li}"], C), C, f"bias{li}_rep") for li in (1, 2)]

            attm = []
            for li in (1, 2):
                t = con.tile([P, 2], F32, tag=f"attm{li}")
                nc.vector.memset(t[:], 0.0)
                nc.sync.dma_start(out=t[0:C, 0:1], in_=w[f"att{li}"][0, :][:, None])
                nc.sync.dma_start(out=t[C:2 * C, 1:2], in_=w[f"att{li}"][1, :][:, None])
                attm.append(t)

            # ---------------- layer-1 tables ----------------
            xr1_res = res.tile([P, BLOCKS, HC], F32, tag="xr1res")
            for j in range(NT):
                xt = sb.tile([128, P], F32, tag="xt")
                nc.sync.dma_start(out=xt[:], in_=xT_d[:, j * P:(j + 1) * P])
                pm = ps_b.tile([P, HC], F32, space="PSUM", tag="ps_mm")
                nc.tensor.matmul(pm[:], lhsT=xt[:], rhs=Wl1_sb[:], start=True, stop=True)
                xlt = sb.tile([P, HC], F32, tag="xlt")
                nc.vector.tensor_add(xlt[:], pm[:], bl1_rep[:])
                nc.sync.dma_start(out=xl1_t[j * P:(j + 1) * P, :], in_=xlt[:])
            for b in range(BLOCKS):
                xt = sb.tile([128, P], F32, tag="xt")
                nc.sync.dma_start(out=xt[:], in_=xTl_d[:, b * P:(b + 1) * P])
                pm = ps_b.tile([P, HC], F32, space="PSUM", tag="ps_mm")
                nc.tensor.matmul(pm[:], lhsT=xt[:], rhs=Wr1_sb[:], start=True, stop=True)
                nc.vector.tensor_add(xr1_res[:, b, :], pm[:], br1_rep[:])

            # ---------------- edge phase (shared for both layers) ----------------
            h1T_res = res.tile([C, NPADC], F32, tag="h1T")
            h2T_res = res.tile([C, NPADC], F32, tag="h2T")

            def edge_layer(li, table, xr_res, hT_res, b_rep):
                pstats = ps_st.tile([C, C + 1], F32, space="PSUM", tag="ps_stats")
                for b in range(BLOCKS):
                    Tb = T_list[b]
                    c0 = sum(T_list[:b])
                    srcg = sidx.tile([P, Tb], I32, tag="srcg")
                    nc.sync.dma_start(out=srcg[:], in_=esrcT_d[:, c0:c0 + Tb])
                    dstg = sidx.tile([P, Tb], F32, tag="dstg")
                    nc.sync.dma_start(out=dstg[:], in_=edstlT_d[:, c0:c0 + Tb])
                    acc = ps_acc.tile([P, HC + 2], F32, space="PSUM", tag="ps_acc")
                    for t in range(Tb):
                        oh = sb.tile([P, P], F32, tag="oh")
                        nc.vector.tensor_tensor(out=oh[:], in0=iota_f[:],
                                                in1=dstg[:, t:t + 1].to_broadcast([P, P]),
                                                op=OP.is_equal)
                        pt = ps_t.tile([P, P], F32, space="PSUM", tag="ps_tr")
                        nc.tensor.transpose(pt[:], oh[:], ident[:])
                        ohT = sb.tile([P, P], F32, tag="ohT")
                        nc.vector.tensor_copy(ohT[:], pt[:])
                        xls = sgath.tile([P, HC], F32, tag="xls")
                        nc.gpsimd.indirect_dma_start(
                            out=xls[:], out_offset=None, in_=table[:],
                            in_offset=bass.IndirectOffsetOnAxis(ap=srcg[:, t:t + 1], axis=0))
                        pb = ps_b.tile([P, P], F32, space="PSUM", tag="ps_mm")
                        nc.tensor.matmul(pb[:], lhsT=xls[:], rhs=ident[:], start=True, stop=False)
                        nc.tensor.matmul(pb[:], lhsT=xr_res[:, b, :], rhs=ohT[:], start=False, stop=True)
                        s02 = sb.tile([P, P], F32, tag="s02")
                        nc.scalar.activation(s02[:], pb[:], AF.Copy, bias=0.0, scale=0.2)
                        r08 = sb.tile([P, P], F32, tag="r08")
                        nc.scalar.activation(r08[:], pb[:], AF.Relu, bias=0.0, scale=0.8)
                        pe = ps_e.tile([2, P], F32, space="PSUM", tag="ps_e")
                        nc.tensor.matmul(pe[:], lhsT=attm[li - 1][:], rhs=s02[:], start=True, stop=False)
                        nc.tensor.matmul(pe[:], lhsT=attm[li - 1][:], rhs=r08[:], start=False, stop=True)
                        eeT = sb.tile([2, P], F32, tag="eeT")
                        nc.scalar.activation(eeT[:], pe[:], AF.Exp)
                        pet = ps_pet.tile([P, 2], F32, space="PSUM", tag="ps_pet")
                        nc.tensor.transpose(pet[:], eeT[:], ident[0:2, 0:2])
                        pay = sb.tile([P, HC + 2], F32, tag="pay")
                        nc.vector.tensor_copy(pay[:, HC:HC + 2], pet[:])
                        nc.vector.tensor_scalar_mul(pay[:, 0:C], xls[:, 0:C], pay[:, HC:HC + 1])
                        nc.vector.tensor_scalar_mul(pay[:, C:HC], xls[:, C:HC], pay[:, HC + 1:HC + 2])
                        nc.tensor.matmul(acc[:], lhsT=oh[:], rhs=pay[:], start=(t == 0), stop=(t == Tb - 1))
                    # ---- drain block b ----
                    last = b == BLOCKS - 1
                    # bias keeps pad-row denominators finite (0 -> 1e-20)
                    d2 = sb.tile([P, 2], F32, tag="d2")
                    nc.scalar.activation(d2[:], acc[:, HC:HC + 2], AF.Copy, bias=1e-20, scale=2.0)
                    rec = sb.tile([P, 2], F32, tag="rec")
                    nc.vector.reciprocal(rec[:], d2[:])
                    t0 = sb.tile([P, C], F32, tag="t0")
                    nc.vector.tensor_scalar_mul(t0[:], acc[:, 0:C], rec[:, 0:1])
                    t1 = sb.tile([P, C], F32, tag="t1")
                    nc.vector.tensor_scalar_mul(t1[:], acc[:, C:HC], rec[:, 1:2])
                    hs = sb.tile([P, C + 1], F32, tag="hs")
                    nc.vector.memset(hs[:, C:C + 1], 1.0)
                    nc.vector.tensor_add(hs[:, 0:C], t0[:], t1[:])
                    hb = hs[:, 0:C]
                    nc.vector.tensor_add(hb, hb, b_rep[:])
                    if last and LAST < P:
                        nc.vector.tensor_scalar_mul(hs[:], hs[:], mask_col[:, 0:1])
                    nc.tensor.matmul(pstats[:], lhsT=hb, rhs=hs[:], start=(b == 0), stop=(b == BLOCKS - 1))
                    pht = ps_t.tile([C, P], F32, space="PSUM", tag="ps_tr")
                    nc.tensor.transpose(pht[:], hb, ident[:])
                    nc.scalar.copy(hT_res[:, b * P:(b + 1) * P], pht[:])
                # ---- stats finalize + AllReduce ----
                trash = sb.tile([C, C], F32, tag="trash")
                st2 = sb.tile([C, 2], F32, tag="st2")
                nc.vector.tensor_mul(trash[:], pstats[:, 0:C], ident[0:C, 0:C])
                nc.vector.tensor_reduce(st2[:, 1:2], trash[:], axis=mybir.AxisListType.X, op=OP.add)
                nc.vector.tensor_copy(st2[:, 0:1], pstats[:, C:C + 1])
                nc.sync.dma_start(out=st_l[li - 1][:], in_=st2[:])
                nc.gpsimd.collective_compute(
                    "AllReduce", OP.add, replica_groups=rg,
                    ins=[st_l[li - 1][:]], outs=[st_g[li - 1][:]])
                stg = sb.tile([C, 2], F32, tag="stg")
                nc.sync.dma_start(out=stg[:], in_=st_g[li - 1][:])
                # A = gng * rsqrt(var+eps); B = gnb - A*a*mean
                a_col = load_col(w[f"gna{li}"], C)
                g_col = load_col(w[f"gng{li}"], C)
                bta_col = load_col(w[f"gnb{li}"], C)
                mean = sb.tile([C, 1], F32, tag="gn_m")
                nc.scalar.activation(mean[:], stg[:, 0:1], AF.Copy, bias=0.0, scale=1.0 / cfg.N)
                msq = sb.tile([C, 1], F32, tag="gn_m2")
                nc.scalar.square(msq[:], mean[:])
                qn = sb.tile([C, 1], F32, tag="gn_qn")
                nc.scalar.activation(qn[:], stg[:, 1:2], AF.Copy, bias=0.0, scale=1.0 / cfg.N)
                a2 = sb.tile([C, 1], F32, tag="gn_a2")
                nc.vector.tensor_mul(a2[:], a_col[:], a_col[:])
                twoa = sb.tile([C, 1], F32, tag="gn_2a")
                nc.scalar.activation(twoa[:], a_col[:], AF.Copy, bias=0.0, scale=2.0)
                coef = sb.tile([C, 1], F32, tag="gn_cf")
                nc.vector.tensor_sub(coef[:], twoa[:], a2[:])
                cm = sb.tile([C, 1], F32, tag="gn_cm")
                nc.vector.tensor_mul(cm[:], coef[:], msq[:])
                var = sb.tile([C, 1], F32, tag="gn_var")
                nc.vector.tensor_sub(var[:], qn[:], cm[:])
                vare = sb.tile([C, 1], F32, tag="gn_vare")
                nc.vector.tensor_scalar_add(vare[:], var[:], cfg.EPS)
                lnv = sb.tile([C, 1], F32, tag="gn_lnv")
                nc.scalar.activation(lnv[:], vare[:], AF.Ln)
                rs = sb.tile([C, 1], F32, tag="gn_rs")
                nc.scalar.activation(rs[:], lnv[:], AF.Exp, bias=0.0, scale=-0.5)
                A = sb.tile([C, 1], F32, tag="gn_A")
                nc.vector.tensor_mul(A[:], g_col[:], rs[:])
                t_ = sb.tile([C, 1], F32, tag="gn_t")
                nc.vector.tensor_mul(t_[:], A[:], a_col[:])
                t2_ = sb.tile([C, 1], F32, tag="gn_t2")
                nc.vector.tensor_mul(t2_[:], t_[:], mean[:])
                B = sb.tile([C, 1], F32, tag="gn_B")
                nc.vector.tensor_sub(B[:], bta_col[:], t2_[:])
                return A, B

            A1, B1 = edge_layer(1, xl1_t, xr1_res, h1T_res, bias_rep[0])

            if DBG:
                nc.sync.dma_start(out=dbg_xl1[:], in_=xl1_t[0:256, :])
                nc.sync.dma_start(out=dbg_h1T[:], in_=h1T_res[:])
                nc.sync.dma_start(out=dbg_st[:], in_=st_g[0][:])
                nc.sync.dma_start(out=dbg_xr1[:], in_=xr1_res[:, 3, :])

            # AllGather h1 (transposed layout)
            nc.sync.dma_start(out=h1T_dr[:], in_=h1T_res[:])
            nc.gpsimd.collective_compute(
                "AllGather", OP.bypass, replica_groups=rg,
                ins=[h1T_dr[:]], outs=[h1T_ag[:]])

            # folded layer-2 weights
            def fold(W_sb, b_d, A, B, ncols, tag):
                Wp = con.tile([C, ncols], F32, tag=f"W_{tag}")
                nc.vector.tensor_scalar_mul(Wp[:], W_sb[:], A[:])
                pbias = ps_b.tile([1, ncols], F32, space="PSUM", tag="ps_mm")
                nc.tensor.matmul(pbias[:], lhsT=B[:], rhs=W_sb[:], start=True, stop=True)
                brow = con.tile([1, ncols], F32, tag=f"brow_{tag}")
                nc.vector.tensor_add(brow[:], pbias[:], load_row(b_d, ncols)[:])
                rep = replicate_row(brow, ncols, f"brep_{tag}")
                return Wp, rep

            Wl2p, bl2p_rep = fold(Wl2_sb, w["bl2"], A1, B1, HC, "l2l")
            Wr2p, br2p_rep = fold(Wr2_sb, w["br2"], A1, B1, HC, "l2r")

            # ---------------- layer-2 tables ----------------
            xr2_res = res.tile([P, BLOCKS, HC], F32, tag="xr2res")
            for j in range(NT):
                k, b = divmod(j, BLOCKS)
                ht = sb.tile([C, P], F32, tag="ht")
                nc.sync.dma_start(out=ht[:], in_=h1T_ag[k * C:(k + 1) * C, b * P:(b + 1) * P])
                pm = ps_b.tile([P, HC], F32, space="PSUM", tag="ps_mm")
                nc.tensor.matmul(pm[:], lhsT=ht[:], rhs=Wl2p[:], start=True, stop=True)
                xlt = sb.tile([P, HC], F32, tag="xlt")
                nc.vector.tensor_add(xlt[:], pm[:], bl2p_rep[:])
                nc.sync.dma_start(out=xl2_t[j * P:(j + 1) * P, :], in_=xlt[:])
            for b in range(BLOCKS):
                pm = ps_b.tile([P, HC], F32, space="PSUM", tag="ps_mm")
                nc.tensor.matmul(pm[:], lhsT=h1T_res[:, b * P:(b + 1) * P], rhs=Wr2p[:], start=True, stop=True)
                nc.vector.tensor_add(xr2_res[:, b, :], pm[:], br2p_rep[:])

            if DBG:
                nc.sync.dma_start(out=dbg_ag[:], in_=h1T_ag[:, 384:512])
                nc.sync.dma_start(out=dbg_xl2[:], in_=xl2_t[0:256, :])
                nc.sync.dma_start(out=dbg_xr2[:], in_=xr2_res[:, 3, :])
                ab = sb.tile([C, 2], F32, tag="dbgab")
                nc.vector.tensor_copy(ab[:, 0:1], A1[:])
                nc.vector.tensor_copy(ab[:, 1:2], B1[:])
                nc.sync.dma_start(out=dbg_AB[:], in_=ab[:])

            A2, B2 = edge_layer(2, xl2_t, xr2_res, h2T_res, bias_rep[1])

            if DBG:
                nc.sync.dma_start(out=dbg_h2T[:], in_=h2T_res[:])

            # ---------------- classifier + log_softmax ----------------
            W1p = con.tile([C, NCLS], F32, tag="W1p")
            nc.vector.tensor_scalar_mul(W1p[:], W1_sb[:], A2[:])
            pb1 = ps_b.tile([1, NCLS], F32, space="PSUM", tag="ps_mm")
            nc.tensor.matmul(pb1[:], lhsT=B2[:], rhs=W1_sb[:], start=True, stop=True)
            b1p = con.tile([1, NCLS], F32, tag="b1p")
            nc.vector.tensor_add(b1p[:], pb1[:], b1_row[:])
            b1p_rep = replicate_row(b1p, NCLS, "b1p_rep")

            for b in range(BLOCKS):
                pl = ps_acc.tile([P, NCLS], F32, space="PSUM", tag="ps_acc")
                nc.tensor.matmul(pl[:], lhsT=h2T_res[:, b * P:(b + 1) * P], rhs=W1p[:], start=True, stop=True)
                lg = sb.tile([P, NCLS], F32, tag="lg")
                nc.vector.tensor_add(lg[:], pl[:], b1p_rep[:])
                mx = sb.tile([P, 1], F32, tag="mx")
                nc.vector.tensor_reduce(mx[:], lg[:], axis=mybir.AxisListType.X, op=OP.max)
                lgm = sb.tile([P, NCLS], F32, tag="lgm")
                nc.vector.tensor_scalar(out=lgm[:], in0=lg[:], scalar1=mx[:, 0:1], scalar2=None, op0=OP.subtract)
                ex = sb.tile([P, NCLS], F32, tag="ex")
                nc.scalar.activation(ex[:], lgm[:], AF.Exp)
                sm = sb.tile([P, 1], F32, tag="sm")
                nc.vector.tensor_reduce(sm[:], ex[:], axis=mybir.AxisListType.X, op=OP.add)
                lns = sb.tile([P, 1], F32, tag="lns")
                nc.scalar.activation(lns[:], sm[:], AF.Ln)
                ot = sb.tile([P, NCLS], F32, tag="ot")
                nc.vector.tensor_scalar(out=ot[:], in0=lgm[:], scalar1=lns[:, 0:1], scalar2=None, op0=OP.subtract)
                rows = min(P, NPC - b * P)
                nc.sync.dma_start(out=out_d[b * P: b * P + rows, :], in_=ot[0:rows, :])

    nc.compile()
    return nc


_CACHE = {}


def _get_program(cfg, T_list):
    key = tuple(T_list)
    if key not in _CACHE:
        _CACHE[key] = _build(cfg, T_list)
    return _CACHE[key]


def _install_axon_ntff_shim():
    """Provide antenv.axon_hooks (missing on this image) so trace=True works
    under axon. Mirrors trn_agent_boot's ctypes hook against libaxon_pjrt.so."""
    import sys, types, ctypes, contextlib, glob as _glob
    try:
        import antenv.axon_hooks  # noqa
        return
    except ImportError:
        pass
    hook = None
    for so_path in (["/opt/axon/libaxon_pjrt.so"] + _glob.glob("/root/.axon_site/**/libaxon_pjrt.so", recursive=True)):
        try:
            lib = ctypes.CDLL(so_path)
        except OSError:
            continue
        if not hasattr(lib, "axon_start_nrt_profile"):
            continue
        lib.axon_start_nrt_profile.argtypes = [ctypes.POINTER(ctypes.c_int64), ctypes.c_size_t]
        lib.axon_start_nrt_profile.restype = ctypes.c_int64
        lib.axon_stop_nrt_profile.argtypes = [ctypes.c_char_p]
        lib.axon_stop_nrt_profile.restype = ctypes.c_int64

        @contextlib.contextmanager
        def _hook(output_dir, device_ids, _lib=lib):
            import jax
            jax.devices()
            if device_ids:
                ids = (ctypes.c_int64 * len(device_ids))(*device_ids)
                rc = _lib.axon_start_nrt_profile(ids, len(device_ids))
            else:
                rc = _lib.axon_start_nrt_profile(None, 0)
            if rc != 0:
                raise RuntimeError(f"axon_start_nrt_profile rc={rc}")
            try:
                yield
            finally:
                n = _lib.axon_stop_nrt_profile(str(output_dir).encode())
                print(f"ntff profile: {n} file(s) -> {output_dir}")

        hook = _hook
        break
    m = types.ModuleType("antenv.axon_hooks")
    m.get_axon_ntff_profile_hook = lambda: hook
    m.set_axon_ntff_profile_hook = lambda h: None
    sys.modules["antenv.axon_hooks"] = m
    try:
        import antenv
        antenv.axon_hooks = m
    except ImportError:
        pass
    # artifact upload has no bucket in this container; keep traces local
    import concourse.bass_utils as bu
    bu.upload_artifacts = lambda tmpdir: str(tmpdir)


def kernel(**inputs):
    from concourse.bass_utils import run_bass_kernel_spmd
    import os

    x = np.ascontiguousarray(np.asarray(inputs["x"], dtype=np.float32))
    edge_index = np.asarray(inputs["edge_index"], dtype=np.int32)
    cfg = Cfg(x.shape[0], 8)
    T_list, per_core = _preprocess(cfg, x, edge_index)
    nc = _get_program(cfg, T_list)

    wnames = []
    for li in (1, 2):
        wnames += [f"Wl{li}", f"bl{li}", f"Wr{li}", f"br{li}", f"att{li}",
                   f"bias{li}", f"gng{li}", f"gnb{li}", f"gna{li}"]
    wnames += ["W1", "b1"]
    base = {}
    for n in wnames:
        a = np.ascontiguousarray(np.asarray(inputs[n], dtype=np.float32))
        if n.startswith(("bl", "br", "bias", "gng", "gnb", "gna", "b1")):
            a = a.reshape(-1)
        base[n] = a
    in_maps = [{**base, **pc} for pc in per_core]

    trace = bool(int(os.environ.get("GAT_TRACE", "0")))
    if trace:
        _install_axon_ntff_shim()
    r = run_bass_kernel_spmd(nc, in_maps, core_ids=list(range(cfg.NC)), trace=trace)
    kernel.last_results = r
    if trace and r.exec_time_ns is not None:
        print(f"HW exec time: {r.exec_time_ns} ns")
        if r.instructions_and_trace is not None:
            print(f"trace: {r.instructions_and_trace[1]}")
        print(f"profile_json: {r.profile_json}")
        kernel.last_exec_ns = r.exec_time_ns
    out = np.concatenate([r.results[k]["out"] for k in range(cfg.NC)], axis=0)
    return out

